# revision 24
# baseline (speedup 1.0000x reference)
"""Trainium2 Bass kernel for a 3x3 'same' conv: x [8,16,512,512] f32, weight [16,144].

Data-parallel over batch: 1 image per NeuronCore, 8 cores.

Measured facts on this machine that drive the design:
  - PE cost is per streamed matmul column (LDWEIGHTS+512-col matmul ~= 350 ns
    warm), so minimize passes: 3 per group (one per kw tap -- each pass has a
    fixed horizontal shift), with row slots for all (r, kh) combos needing
    16*(R+2) <= 128 partitions => R=6 output rows per group, 86 groups,
    258 matmuls ~= 90 us PE span. fp32r matmuls stream at half clock -- use
    fp16 operands (fp32 PSUM accumulation) instead.
  - DMA time scales with bytes moved (~14-23 GB/s per SDMA engine, 16
    engines); fp16 x + fp16 out nearly halves bytes vs fp32 at ~1e-4 cost.

Modes (CONV_MODE env):
  f16  (default): x/weights fp16, fp32 PSUM accumulate + fp32 output.
  f16o: fp16 output too (host upcasts); fewest bytes, adds ~2.4e-4 rounding.
  f32r: all-fp32 (relaxed-precision fp32r matmul); most accurate (~1.5e-4).

Structure (all modes):
  - Host pads x columns to [16, 512, 514] with zero cols 0 and 513 so the
    horizontal taps become plain SBUF column offsets (no device memsets,
    full-bank PSUM writes -- fp32r's dst-pattern ISA restriction).
  - Group g covers output rows [y0, y0+6); its x-tile holds the 8-row padded
    window at partition p = ci*8 + j (row Y+j, Y = clamp(y0-1, 0, 504)),
    K = 128. Three accumulating matmuls (kw = 0,1,2; rhs columns [kw, kw+512))
    into one PSUM bank [96, 512] (M = 16 co x 6 rows).
  - Stationary weights per (kw, boundary variant b): [128, 96] matrices
    wk[ci*8+j, co*6+r] = w[co, ci, j-r-(b-1), kw]; entries whose target row
    falls outside the window are dropped (those are the zero-pad rows).
  - PSUM -> SBUF via VectorE copy; input DMAs on the sync HWDGE queue,
    output DMAs on the scalar HWDGE queue.
"""

import os
from contextlib import ExitStack

import numpy as np

C_OUT, C_IN, KH, KW = 16, 16, 3, 3
H = W = 512
WP = W + 2      # host-padded row length
B = 8
R = 6           # output rows per group
J = R + 2      # input rows per group
M = C_OUT * R   # 96 psum partitions
K = C_IN * J    # 128 contraction partitions
NV = KW * 3     # stationary variants: kw x boundary
GROUP_Y0 = [6 * g for g in range(85)] + [506]

MODE = os.environ.get("CONV_MODE", "f16o")  # f16o | f16 | f32r

_CACHE = {}


def _build_weights(weight: np.ndarray) -> np.ndarray:
    """[16,144] -> [128, 9*96] stationary matrices, variant v = kw*3 + b.

    wk[ci*J+j, v, co*R+r] = w[co, ci, kh, kw] at j = r + kh + (b-1); (r, kh)
    with j outside [0, J) dropped (they reference the zero-pad rows).
    """
    w = np.asarray(weight, dtype=np.float32).reshape(C_OUT, C_IN, KH, KW)
    wk = np.zeros((KW, 3, K, M), np.float32)
    for kw in range(KW):
        for b in range(3):
            for co in range(C_OUT):
                for r in range(R):
                    for kh in range(KH):
                        j = r + kh + (b - 1)
                        if 0 <= j < J:
                            for ci in range(C_IN):
                                wk[kw, b, ci * J + j, co * R + r] = w[co, ci, kh, kw]
    out = np.ascontiguousarray(wk.transpose(2, 0, 1, 3).reshape(K, NV * M))
    return out if MODE == "f32r" else out.astype(np.float16)


def _build_nc():
    import concourse.tile as tile
    from concourse import bacc, mybir

    f32 = mybir.dt.float32
    dt_in = mybir.dt.float32r if MODE == "f32r" else mybir.dt.float16
    dt_out = mybir.dt.float16 if MODE == "f16o" else f32

    nc = bacc.Bacc("TRN2", target_bir_lowering=False, debug=False,
                   enable_asserts=False, num_devices=B)
    # for f32r, declaring inputs as the matmul dtype keeps the BIR fp32r
    # producer->consumer chain consistent (same 4-byte layout as float32)
    x = nc.dram_tensor("x", [C_IN, H, WP], dt_in, kind="ExternalInput").ap()
    wkin = nc.dram_tensor("wk", [K, NV * M], dt_in, kind="ExternalInput").ap()
    out = nc.dram_tensor("out", [C_OUT, H, W], dt_out, kind="ExternalOutput").ap()

    with tile.TileContext(nc) as tc, ExitStack() as ctx:
        wpool = ctx.enter_context(tc.tile_pool(name="wpool", bufs=1))
        xpool = ctx.enter_context(tc.tile_pool(name="xpool", bufs=20))
        opool = ctx.enter_context(tc.tile_pool(name="opool", bufs=14))
        ppool = ctx.enter_context(tc.tile_pool(name="ppool", bufs=8, space="PSUM"))

        wt = wpool.tile([K, NV * M], dt_in, name="wt")
        # per-variant weight loads so the first group's stationaries land
        # quickly (first matmul needs only v=0)
        for v in range(NV):
            eng = nc.scalar if v % 2 == 0 else nc.sync
            eng.dma_start(out=wt[:, v * M : (v + 1) * M],
                          in_=wkin[:, v * M : (v + 1) * M])

        # batches of 4 groups, matmuls issued kw-major (4 consecutive
        # matmuls share one stationary)
        GB = 4
        for i0 in range(0, len(GROUP_Y0), GB):
            batch = GROUP_Y0[i0 : i0 + GB]
            bs = [0 if y0 == 0 else (2 if y0 == H - R else 1) for y0 in batch]

            xts, pts = [], []
            for y0 in batch:
                Y = min(max(y0 - 1, 0), H - J)
                xtile = xpool.tile([K, WP], dt_in, name="xtile", tag="xtile")
                nc.sync.dma_start(out=xtile[:], in_=x[:, Y : Y + J, :])
                xts.append(xtile)
                pts.append(ppool.tile([M, W], f32, name="pt", tag="pt"))

            for kw in range(KW):
                for xtile, pt, b in zip(xts, pts, bs):
                    v = kw * 3 + b
                    nc.tensor.matmul(pt[:, 0:W], wt[:, v * M : (v + 1) * M],
                                     xtile[:, kw : kw + W],
                                     start=(kw == 0), stop=(kw == KW - 1))

            for y0, pt in zip(batch, pts):
                ot = opool.tile([M, W], dt_out, name="ot", tag="ot")
                nc.vector.tensor_copy(ot[:], pt[:])
                nc.scalar.dma_start(out=out[:, y0 : y0 + R, :], in_=ot[:])

    nc.compile()
    return nc


def get_nc():
    if MODE not in _CACHE:
        _CACHE[MODE] = _build_nc()
    return _CACHE[MODE]


def run(x: np.ndarray, weight: np.ndarray, **spmd_kwargs):
    """Run the conv on 8 cores; returns (out [8,16,512,512] f32, results)."""
    from concourse.bass_utils import run_bass_kernel_spmd

    x = np.asarray(x, dtype=np.float32)
    np_in = np.float32 if MODE == "f32r" else np.float16
    xp = np.zeros((B, C_IN, H, WP), np_in)
    xp[:, :, :, 1 : W + 1] = x.astype(np_in)
    wk = _build_weights(weight)
    nc = get_nc()
    in_maps = [{"x": xp[b], "wk": wk} for b in range(B)]
    res = run_bass_kernel_spmd(nc, in_maps, list(range(B)), **spmd_kwargs)
    out = np.stack([res.results[b]["out"] for b in range(B)], axis=0)
    if out.dtype != np.float32:
        out = out.astype(np.float32)
    return out, res


def kernel(x: np.ndarray, weight: np.ndarray) -> np.ndarray:
    return run(x, weight)[0]


# revision 25
# speedup vs baseline: 1.0374x; 1.0374x over previous
"""Trainium2 Bass kernel for a 3x3 'same' conv: x [8,16,512,512] f32, weight [16,144].

Data-parallel over batch: 1 image per NeuronCore, 8 cores.

Measured facts on this machine that drive the design:
  - PE cost is per streamed matmul column (LDWEIGHTS+512-col matmul ~= 350 ns
    warm), so minimize passes: 3 per group (one per kw tap -- each pass has a
    fixed horizontal shift), with row slots for all (r, kh) combos needing
    16*(R+2) <= 128 partitions => R=6 output rows per group, 86 groups,
    258 matmuls ~= 90 us PE span. fp32r matmuls stream at half clock -- use
    fp16 operands (fp32 PSUM accumulation) instead.
  - DMA time scales with bytes moved (~14-23 GB/s per SDMA engine, 16
    engines); fp16 x + fp16 out nearly halves bytes vs fp32 at ~1e-4 cost.

Modes (CONV_MODE env):
  f16  (default): x/weights fp16, fp32 PSUM accumulate + fp32 output.
  f16o: fp16 output too (host upcasts); fewest bytes, adds ~2.4e-4 rounding.
  f32r: all-fp32 (relaxed-precision fp32r matmul); most accurate (~1.5e-4).

Structure (all modes):
  - Host pads x columns to [16, 512, 514] with zero cols 0 and 513 so the
    horizontal taps become plain SBUF column offsets (no device memsets,
    full-bank PSUM writes -- fp32r's dst-pattern ISA restriction).
  - Group g covers output rows [y0, y0+6); its x-tile holds the 8-row padded
    window at partition p = ci*8 + j (row Y+j, Y = clamp(y0-1, 0, 504)),
    K = 128. Three accumulating matmuls (kw = 0,1,2; rhs columns [kw, kw+512))
    into one PSUM bank [96, 512] (M = 16 co x 6 rows).
  - Stationary weights per (kw, boundary variant b): [128, 96] matrices
    wk[ci*8+j, co*6+r] = w[co, ci, j-r-(b-1), kw]; entries whose target row
    falls outside the window are dropped (those are the zero-pad rows).
  - PSUM -> SBUF via VectorE copy; input DMAs on the sync HWDGE queue,
    output DMAs on the scalar HWDGE queue.
"""

import os
from contextlib import ExitStack

import numpy as np

C_OUT, C_IN, KH, KW = 16, 16, 3, 3
H = W = 512
WP = W + 2      # host-padded row length
B = 8
R = 6           # output rows per group
J = R + 2      # input rows per group
M = C_OUT * R   # 96 psum partitions
K = C_IN * J    # 128 contraction partitions
NV = KW * 3     # stationary variants: kw x boundary
GROUP_Y0 = [6 * g for g in range(85)] + [506]

MODE = os.environ.get("CONV_MODE", "f16o")  # f16o | f16 | f32r

_CACHE = {}


def _build_weights(weight: np.ndarray) -> np.ndarray:
    """[16,144] -> [128, 9*96] stationary matrices, variant v = kw*3 + b.

    wk[ci*J+j, v, co*R+r] = w[co, ci, kh, kw] at j = r + kh + (b-1); (r, kh)
    with j outside [0, J) dropped (they reference the zero-pad rows).
    """
    w = np.asarray(weight, dtype=np.float32).reshape(C_OUT, C_IN, KH, KW)
    wk = np.zeros((KW, 3, K, M), np.float32)
    for kw in range(KW):
        for b in range(3):
            for co in range(C_OUT):
                for r in range(R):
                    for kh in range(KH):
                        j = r + kh + (b - 1)
                        if 0 <= j < J:
                            for ci in range(C_IN):
                                wk[kw, b, ci * J + j, co * R + r] = w[co, ci, kh, kw]
    out = np.ascontiguousarray(wk.transpose(2, 0, 1, 3).reshape(K, NV * M))
    return out if MODE == "f32r" else out.astype(np.float16)


def _build_nc():
    import concourse.tile as tile
    from concourse import bacc, mybir

    f32 = mybir.dt.float32
    dt_in = mybir.dt.float32r if MODE == "f32r" else mybir.dt.float16
    dt_out = mybir.dt.float16 if MODE == "f16o" else f32

    nc = bacc.Bacc("TRN2", target_bir_lowering=False, debug=False,
                   enable_asserts=False, num_devices=B)
    # for f32r, declaring inputs as the matmul dtype keeps the BIR fp32r
    # producer->consumer chain consistent (same 4-byte layout as float32)
    x = nc.dram_tensor("x", [C_IN, H, WP], dt_in, kind="ExternalInput").ap()
    wkin = nc.dram_tensor("wk", [K, NV * M], dt_in, kind="ExternalInput").ap()
    out = nc.dram_tensor("out", [C_OUT, H, W], dt_out, kind="ExternalOutput").ap()

    with tile.TileContext(nc) as tc, ExitStack() as ctx:
        wpool = ctx.enter_context(tc.tile_pool(name="wpool", bufs=1))
        xpool = ctx.enter_context(tc.tile_pool(name="xpool", bufs=12))
        opool = ctx.enter_context(tc.tile_pool(name="opool", bufs=10))
        ppool = ctx.enter_context(tc.tile_pool(name="ppool", bufs=8, space="PSUM"))

        wt = wpool.tile([K, NV * M], dt_in, name="wt")
        # per-variant weight loads so the first group's stationaries land
        # quickly (first matmul needs only v=0)
        for v in range(NV):
            eng = nc.scalar if v % 2 == 0 else nc.sync
            eng.dma_start(out=wt[:, v * M : (v + 1) * M],
                          in_=wkin[:, v * M : (v + 1) * M])

        # batches of 4 groups, matmuls issued kw-major (4 consecutive
        # matmuls share one stationary)
        GB = 4
        for i0 in range(0, len(GROUP_Y0), GB):
            batch = GROUP_Y0[i0 : i0 + GB]
            bs = [0 if y0 == 0 else (2 if y0 == H - R else 1) for y0 in batch]

            xts, pts = [], []
            for y0 in batch:
                Y = min(max(y0 - 1, 0), H - J)
                xtile = xpool.tile([K, WP], dt_in, name="xtile", tag="xtile")
                nc.sync.dma_start(out=xtile[:], in_=x[:, Y : Y + J, :])
                xts.append(xtile)
                pts.append(ppool.tile([M, W], f32, name="pt", tag="pt"))

            for kw in range(KW):
                for xtile, pt, b in zip(xts, pts, bs):
                    v = kw * 3 + b
                    nc.tensor.matmul(pt[:, 0:W], wt[:, v * M : (v + 1) * M],
                                     xtile[:, kw : kw + W],
                                     start=(kw == 0), stop=(kw == KW - 1))

            for y0, pt in zip(batch, pts):
                ot = opool.tile([M, W], dt_out, name="ot", tag="ot")
                nc.vector.tensor_copy(ot[:], pt[:])
                nc.scalar.dma_start(out=out[:, y0 : y0 + R, :], in_=ot[:])

    nc.compile()
    return nc


def get_nc():
    if MODE not in _CACHE:
        _CACHE[MODE] = _build_nc()
    return _CACHE[MODE]


def run(x: np.ndarray, weight: np.ndarray, **spmd_kwargs):
    """Run the conv on 8 cores; returns (out [8,16,512,512] f32, results)."""
    from concourse.bass_utils import run_bass_kernel_spmd

    x = np.asarray(x, dtype=np.float32)
    np_in = np.float32 if MODE == "f32r" else np.float16
    xp = np.zeros((B, C_IN, H, WP), np_in)
    xp[:, :, :, 1 : W + 1] = x.astype(np_in)
    wk = _build_weights(weight)
    nc = get_nc()
    in_maps = [{"x": xp[b], "wk": wk} for b in range(B)]
    res = run_bass_kernel_spmd(nc, in_maps, list(range(B)), **spmd_kwargs)
    out = np.stack([res.results[b]["out"] for b in range(B)], axis=0)
    if out.dtype != np.float32:
        out = out.astype(np.float32)
    return out, res


def kernel(x: np.ndarray, weight: np.ndarray) -> np.ndarray:
    return run(x, weight)[0]
